# revision 1
# baseline (speedup 1.0000x reference)
"""Chamfer distance loss kernel for 8 Trainium2 NeuronCores.

Problem: points1 [8, 4096, 3], points2 [8, 4096, 3] (f32).
  dist[b,n,m] = ||p1[b,n]||^2 + ||p2[b,m]||^2 - 2 p1.p2
  loss = (mean_n,b(min_m dist) + mean_m,b(min_n dist)) / 8     (scalar f32)

Sharding: data-parallel over batch B: core b handles batch b.

Per-core algorithm (flash-style, nothing materialized in HBM):
  Host lifts each point cloud to K=8 rows so that the *negated* distance
  matrix is one K=8 matmul:  -d[n,m] = sum_k la[k,n] * lb[k,m]
     la[:,n] = [sq1[n], 1, x1, y1, z1, 0,0,0]
     lb[:,m] = [-1, -sq2[m], 2*x2, 2*y2, 2*z2, 0,0,0]
  (negated so every reduction is a MAX - gpsimd partition_all_reduce has
   max but not min)
  Device loop over 32 row-strips of 128 points1 (processed in groups of 4):
     PE:  8 matmuls (N=512, fp32, 4-way row-group packed via tile_position)
          -> PSUM strip [128, 4096] f32 (2 halves)
     ACT: cast PSUM f32 -> SBUF fp16 strip
     DVE: colacc = max(colacc, strip) elementwise (fp16 2x mode)
          rowmax[n] via a fold-max tree 4096->128, one 3D-AP op per level
          covering the whole 4-strip group (amortizes per-op overheads)
  Tail: colacc partition-max via 32 PE transposes (f16 PSUM) + DVE block
        reduces, fused sum, one f32 scalar ( -(rowsum+colsum) ) DMA'd out.
Host: loss = -sum(partials) / (B*B*N).
"""

import sys
import numpy as np

for _p in ("/opt/trn_rl_repo", "/root/.axon_site/_ro/trn_rl_repo"):
    if _p not in sys.path:
        sys.path.insert(0, _p)

B = 8
N = 4096
D = 3
K = 8
P = 128
NSTRIP = N // P          # 32
MM_FREE = 512            # fp32 matmul moving-operand max
MHALF = 2048             # half strip (4 PSUM banks)

_NC_CACHE = {}


def _build_nc(repeat=1, packed=True, gsplit=0, group=4, maskred=False):
    """Build the per-core bass program.

    repeat: wrap the whole compute body in an on-device For_i loop (used
        only for timing: slope over `repeat` isolates device time from the
        ~5ms axon launch overhead).
    packed: pack 4 concurrent K=8 matmuls into PE row-groups 0/32/64/96
        (fp32 matmuls run at 4 cycles/row; packing restores ~1 cycle/row).
    gsplit: unused (GPSIMD software tensor_tensor(max) and DMA CCE max are
        not supported by this toolchain; kept for API compat).
    """
    import contextlib

    import concourse.bacc as bacc
    import concourse.tile as tile
    from concourse import bass_isa, mybir

    F16 = mybir.dt.float16
    F32 = mybir.dt.float32
    MAX = mybir.AluOpType.max
    ADD = mybir.AluOpType.add

    nc = bacc.Bacc(
        "TRN2", target_bir_lowering=False, debug=False, num_devices=B
    )
    la = nc.declare_dram_parameter("la", [K, N], F32, isOutput=False)
    lb = nc.declare_dram_parameter("lb", [K, N], F32, isOutput=False)
    ident = nc.declare_dram_parameter("ident", [P, P], F16, isOutput=False)
    out = nc.declare_dram_parameter("partial", [1, 1], F32, isOutput=True)

    with tile.TileContext(nc) as tc:
        with (
            tc.tile_pool(name="consts", bufs=1) as consts,
            tc.tile_pool(name="strips", bufs=3 if group <= 2 else 2) as strips,
            tc.tile_pool(name="scr", bufs=2) as scr,
            tc.tile_pool(name="accs", bufs=1) as accs,
            tc.tile_pool(name="psum", bufs=2, space="PSUM") as psum,
        ):
            if packed:
                # 4 copies of the lifted tensors at partition offsets
                # 0/32/64/96 so 4 matmuls can run in distinct PE row-groups.
                la_sb = consts.tile([3 * 32 + K, N], F32)
                lb_sb = consts.tile([3 * 32 + K, N], F32)
                # parallel input load: la on the SP HWDGE queue, lb on the
                # Activation HWDGE queue (the only two HWDGE engines)
                for q in range(4):
                    nc.sync.dma_start(out=la_sb[32 * q : 32 * q + K, :], in_=la[:])
                    nc.scalar.dma_start(out=lb_sb[32 * q : 32 * q + K, :], in_=lb[:])
            else:
                la_sb = consts.tile([K, N], F32)
                lb_sb = consts.tile([K, N], F32)
                nc.sync.dma_start(out=la_sb[:], in_=la[:])
                nc.sync.dma_start(out=lb_sb[:], in_=lb[:])
            idt = consts.tile([P, P], F16)
            nc.gpsimd.dma_start(out=idt[:], in_=ident[:])

            loop_ctx = (
                tc.For_i(0, repeat, 1) if repeat != 1 else contextlib.nullcontext()
            )
            with loop_ctx:
                colacc = accs.tile([P, N], F16)
                # per-strip partially-folded rowmax candidates (128 per strip)
                rowacc = accs.tile([P, NSTRIP * 128], F16)
                summ = accs.tile([P, 2 * NSTRIP], F32)
                if maskred:
                    mask_n = accs.tile([P, 1], F32)
                    nc.vector.memset(mask_n[:], float(N))

                def emit_mms(i, h, ph):
                    for j in range(MHALF // MM_FREE):
                        m0 = j * MM_FREE
                        if packed:
                            nc.tensor.matmul(
                                ph[:, m0 : m0 + MM_FREE],
                                lhsT=la_sb[32 * j : 32 * j + K, i * P : (i + 1) * P],
                                rhs=lb_sb[
                                    32 * j : 32 * j + K,
                                    h * MHALF + m0 : h * MHALF + m0 + MM_FREE,
                                ],
                                start=True,
                                stop=True,
                                tile_position=(32 * j, 0),
                            )
                        else:
                            nc.tensor.matmul(
                                ph[:, m0 : m0 + MM_FREE],
                                lhsT=la_sb[:, i * P : (i + 1) * P],
                                rhs=lb_sb[
                                    :, h * MHALF + m0 : h * MHALF + m0 + MM_FREE
                                ],
                                start=True,
                                stop=True,
                            )

                if group > 1:
                    # `group` strips per iteration; fold ops span the whole
                    # group via 3D APs, dividing DVE per-op overheads
                    G = group
                    for ip in range(NSTRIP // G):
                        dstrip = strips.tile([P, G, N], F16, tag="strip")
                        last_sub = ip == NSTRIP // G - 1
                        for s in range(G):
                            i = G * ip + s
                            for h in range(2):
                                ph = psum.tile([P, MHALF], F32, tag="ph")
                                emit_mms(i, h, ph)
                                nc.scalar.copy(
                                    dstrip[:, s, h * MHALF : (h + 1) * MHALF], ph[:]
                                )
                            if ip == 0 and s == 0:
                                # first strip initializes colacc (tensor_copy
                                # runs in the 4x DVE mode, and this replaces
                                # a memset + max)
                                nc.vector.tensor_copy(colacc[:], dstrip[:, s, :])
                            elif last_sub and s == G - 1:
                                # final colmax split by m-quarters so the
                                # tail's PE transposes can start per-range
                                for q in range(4):
                                    qs = q * (N // 4)
                                    qe = qs + N // 4
                                    nc.vector.tensor_tensor(
                                        colacc[:, qs:qe],
                                        colacc[:, qs:qe],
                                        dstrip[:, s, qs:qe],
                                        op=MAX,
                                    )
                            else:
                                nc.vector.tensor_tensor(
                                    colacc[:], colacc[:], dstrip[:, s, :], op=MAX
                                )
                        if maskred:
                            for s2 in range(G):
                                mscr = scr.tile([P, N], F16, tag="mscr")
                                nc.vector.tensor_mask_reduce(
                                    out=mscr[:],
                                    in_=dstrip[:, s2, :],
                                    mask_start=0.0,
                                    mask_end=mask_n[:],
                                    scale=1.0,
                                    accum_in=-1.0e30,
                                    op=MAX,
                                    accum_out=summ[
                                        :, G * ip + s2 : G * ip + s2 + 1
                                    ],
                                )
                        else:
                            w = N // 2
                            src = dstrip
                            while w > 128:
                                dst = scr.tile([P, G, w], F16, tag=f"fold{w}")
                                nc.vector.tensor_tensor(
                                    dst[:], src[:, :, 0:w], src[:, :, w : 2 * w], op=MAX
                                )
                                src = dst
                                w //= 2
                            nc.vector.tensor_tensor(
                                rowacc[:, G * ip * 128 : (G * ip + G) * 128].rearrange(
                                    "p (s w) -> p s w", s=G
                                ),
                                src[:, :, 0:128],
                                src[:, :, 128:256],
                                op=MAX,
                            )
                else:
                    for i in range(NSTRIP):
                        strip = strips.tile([P, N], F16, tag="strip")
                        for h in range(2):
                            ph = psum.tile([P, MHALF], F32, tag="ph")
                            emit_mms(i, h, ph)
                            # cast f32 PSUM -> f16 SBUF (ScalarE/ACT)
                            nc.scalar.copy(
                                strip[:, h * MHALF : (h + 1) * MHALF], ph[:]
                            )
                        # running elementwise colmax
                        nc.vector.tensor_tensor(
                            colacc[:], colacc[:], strip[:], op=MAX
                        )
                        # rowmax fold chain 4096 -> 128 (fp16 TT keeps 2x mode)
                        w = N // 2
                        src = strip
                        while w > 128:
                            dst = scr.tile([P, w], F16, tag=f"fold{w}")
                            nc.vector.tensor_tensor(
                                dst[:], src[:, 0:w], src[:, w : 2 * w], op=MAX
                            )
                            src = dst
                            w //= 2
                        nc.vector.tensor_tensor(
                            rowacc[:, i * 128 : (i + 1) * 128],
                            src[:, 0:128],
                            src[:, 128:256],
                            op=MAX,
                        )

                # ---- tail ----
                # summ[:, 0:32]  = per-(partition, strip) rowmax
                # summ[:, 32:64] = per-(partition, block) colmax via PE transpose
                # per-strip rowmax: fold the 128 candidates per strip down to
                # 2 at 2x mode, then one small 1x reduce
                rw = 0 if maskred else 64
                v = rowacc[:].rearrange("p (i w) -> p i w", w=128)
                while rw >= 2:
                    rdst = scr.tile([P, NSTRIP, rw], F16, tag=f"rfold{rw}")
                    nc.vector.tensor_tensor(
                        rdst[:], v[:, :, 0:rw], v[:, :, rw : 2 * rw], op=MAX
                    )
                    v = rdst[:]
                    rw //= 2
                if not maskred:
                    nc.vector.tensor_reduce(
                        out=summ[:, 0:NSTRIP],
                        in_=v,
                        axis=mybir.AxisListType.X,
                        op=MAX,
                    )
                for k in range(NSTRIP):
                    tp = psum.tile([P, P], F16, tag="ph")
                    nc.tensor.transpose(tp[:], colacc[:, k * P : (k + 1) * P], idt[:])
                    nc.vector.tensor_reduce(
                        out=summ[:, NSTRIP + k : NSTRIP + k + 1],
                        in_=tp[:],
                        axis=mybir.AxisListType.X,
                        op=MAX,
                    )
                tot = accs.tile([P, 1], F32)
                nc.vector.tensor_reduce(
                    out=tot[:], in_=summ[:], axis=mybir.AxisListType.X, op=ADD
                )
                tot_red = accs.tile([P, 1], F32)
                nc.gpsimd.partition_all_reduce(
                    tot_red[:], tot[:], P, bass_isa.ReduceOp.add
                )
                nc.sync.dma_start(out=out[:], in_=tot_red[0:1, :])

    nc.compile()
    return nc


def get_nc(repeat=1, packed=True, gsplit=0, group=4, maskred=False):
    key = (repeat, packed, gsplit, group, maskred)
    if key not in _NC_CACHE:
        _NC_CACHE[key] = _build_nc(
            repeat=repeat, packed=packed, gsplit=gsplit, group=group,
            maskred=maskred,
        )
    return _NC_CACHE[key]


def _lift(points1, points2):
    """Host-side O(N) prep: lifted vectors so -dist = la^T @ lb."""
    p1 = np.asarray(points1, dtype=np.float32)
    p2 = np.asarray(points2, dtype=np.float32)
    sq1 = np.sum(p1 * p1, axis=-1)  # [B, N]
    sq2 = np.sum(p2 * p2, axis=-1)  # [B, N]
    la = np.zeros((B, K, N), dtype=np.float32)
    lb = np.zeros((B, K, N), dtype=np.float32)
    la[:, 0, :] = sq1
    la[:, 1, :] = 1.0
    la[:, 2:5, :] = np.transpose(p1, (0, 2, 1))
    lb[:, 0, :] = -1.0
    lb[:, 1, :] = -sq2
    lb[:, 2:5, :] = 2.0 * np.transpose(p2, (0, 2, 1))
    return la, lb


def _in_maps(points1, points2):
    la, lb = _lift(points1, points2)
    ident = np.eye(P, dtype=np.float16)
    return [
        {
            "la": np.ascontiguousarray(la[b]),
            "lb": np.ascontiguousarray(lb[b]),
            "ident": ident,
        }
        for b in range(B)
    ]


def kernel(points1, points2):
    from concourse.bass_utils import run_bass_kernel_spmd

    in_maps = _in_maps(points1, points2)
    nc = get_nc()
    res = run_bass_kernel_spmd(nc, in_maps, list(range(B))).results
    tot = -sum(float(res[b]["partial"][0, 0]) for b in range(B))
    loss = tot / (B * B * N)
    return np.float32(loss)



# revision 5
# speedup vs baseline: 1.0285x; 1.0285x over previous
"""Chamfer distance loss kernel for 8 Trainium2 NeuronCores.

Problem: points1 [8, 4096, 3], points2 [8, 4096, 3] (f32).
  dist[b,n,m] = ||p1[b,n]||^2 + ||p2[b,m]||^2 - 2 p1.p2
  loss = (mean_n,b(min_m dist) + mean_m,b(min_n dist)) / 8     (scalar f32)

Sharding: data-parallel over batch B: core b handles batch b.

Per-core algorithm (flash-style, nothing materialized in HBM):
  Host lifts each point cloud to K=8 rows so that the *negated* distance
  matrix is one K=8 matmul:  -d[n,m] = sum_k la[k,n] * lb[k,m]
     la[:,n] = [sq1[n], 1, x1, y1, z1, 0,0,0]
     lb[:,m] = [-1, -sq2[m], 2*x2, 2*y2, 2*z2, 0,0,0]
  (negated so every reduction is a MAX)
  Device loop over 32 row-strips of 128 points1 (groups of 4):
     PE:  8 matmuls (N=512, fp32, 4-way row-group packed via tile_position)
          -> PSUM strip [128, 4096] f32 (2 halves)
     ACT: cast PSUM f32 -> SBUF fp16 strip (strip 0 casts straight into
          colacc, replacing a DVE init copy)
     DVE: colacc = max(colacc, strip) elementwise (fp16 2x mode)
          rowmax[n] via a fold-max tree 4096->128, one 3D-AP op per level
          covering the whole 4-strip group (amortizes per-op overheads)
  Tail: rowacc fold chain -> summ[:,0:32]
        colacc partition-max via PE transposes batched 8-per-PSUM-bank +
        one grouped 3D tensor_reduce per bank (4 DVE ops instead of 32)
        summ add-reduce -> [128,1], partition-sum via a PE ones-matmul
        (replaces gpsimd.partition_all_reduce), one f32 scalar DMA'd out.
Host: loss = -sum(partials) / (B*B*N).
"""

import sys
import numpy as np

for _p in ("/opt/trn_rl_repo", "/root/.axon_site/_ro/trn_rl_repo"):
    if _p not in sys.path:
        sys.path.insert(0, _p)

B = 8
N = 4096
D = 3
K = 8
P = 128
NSTRIP = N // P          # 32
MM_FREE = 512            # fp32 matmul moving-operand max
MHALF = 2048             # half strip (4 PSUM banks)

_NC_CACHE = {}


def _build_nc(repeat=1, packed=True, gsplit=0, group=4, maskred=False):
    """Build the per-core bass program.

    repeat: wrap the whole compute body in an on-device For_i loop (used
        only for timing: slope over `repeat` isolates device time from the
        ~5ms axon launch overhead).
    packed: pack 4 concurrent K=8 matmuls into PE row-groups 0/32/64/96
        (fp32 matmuls run at 4 cycles/row; packing restores ~1 cycle/row).
    gsplit/maskred: unused, kept for API compat.
    """
    import contextlib

    import concourse.bacc as bacc
    import concourse.tile as tile
    from concourse import mybir

    F16 = mybir.dt.float16
    F32 = mybir.dt.float32
    MAX = mybir.AluOpType.max
    ADD = mybir.AluOpType.add

    nc = bacc.Bacc(
        "TRN2", target_bir_lowering=False, debug=False, num_devices=B
    )
    la = nc.declare_dram_parameter("la", [K, N], F32, isOutput=False)
    lb = nc.declare_dram_parameter("lb", [K, N], F32, isOutput=False)
    ident = nc.declare_dram_parameter("ident", [P, P], F16, isOutput=False)
    out = nc.declare_dram_parameter("partial", [1, 1], F32, isOutput=True)

    with tile.TileContext(nc) as tc:
        with (
            tc.tile_pool(name="consts", bufs=1) as consts,
            tc.tile_pool(name="strips", bufs=3 if group <= 2 else 2) as strips,
            tc.tile_pool(name="scr", bufs=2) as scr,
            tc.tile_pool(name="accs", bufs=1) as accs,
            tc.tile_pool(name="psum", bufs=2, space="PSUM") as psum,
        ):
            if packed:
                # 4 copies of the lifted tensors at partition offsets
                # 0/32/64/96 so 4 matmuls can run in distinct PE row-groups.
                la_sb = consts.tile([3 * 32 + K, N], F32)
                lb_sb = consts.tile([3 * 32 + K, N], F32)
                for q in range(4):
                    nc.sync.dma_start(out=la_sb[32 * q : 32 * q + K, :], in_=la[:])
                    nc.scalar.dma_start(out=lb_sb[32 * q : 32 * q + K, :], in_=lb[:])
            else:
                la_sb = consts.tile([K, N], F32)
                lb_sb = consts.tile([K, N], F32)
                nc.sync.dma_start(out=la_sb[:], in_=la[:])
                nc.sync.dma_start(out=lb_sb[:], in_=lb[:])
            idt = consts.tile([P, P], F16)
            nc.gpsimd.dma_start(out=idt[:], in_=ident[:])
            ones = consts.tile([P, 1], F32)
            nc.gpsimd.memset(ones[:], 1.0)

            loop_ctx = (
                tc.For_i(0, repeat, 1) if repeat != 1 else contextlib.nullcontext()
            )
            with loop_ctx:
                colacc = accs.tile([P, N], F16)
                # per-strip partially-folded rowmax candidates (128 per strip)
                rowacc = accs.tile([P, NSTRIP * 128], F16)
                summ = accs.tile([P, 2 * NSTRIP], F32)

                def emit_mms(i, h, ph):
                    for j in range(MHALF // MM_FREE):
                        m0 = j * MM_FREE
                        if packed:
                            nc.tensor.matmul(
                                ph[:, m0 : m0 + MM_FREE],
                                lhsT=la_sb[32 * j : 32 * j + K, i * P : (i + 1) * P],
                                rhs=lb_sb[
                                    32 * j : 32 * j + K,
                                    h * MHALF + m0 : h * MHALF + m0 + MM_FREE,
                                ],
                                start=True,
                                stop=True,
                                tile_position=(32 * j, 0),
                            )
                        else:
                            nc.tensor.matmul(
                                ph[:, m0 : m0 + MM_FREE],
                                lhsT=la_sb[:, i * P : (i + 1) * P],
                                rhs=lb_sb[
                                    :, h * MHALF + m0 : h * MHALF + m0 + MM_FREE
                                ],
                                start=True,
                                stop=True,
                            )

                def fold_tree(src_3d, g, out_slice):
                    """Max-fold [P, g, N] (3D AP view) down to 128 per strip,
                    writing [P, g*128] into rowacc[out_slice]. Tags shared
                    across g so all call sites reuse the same scratch slots."""
                    w = N // 2
                    src = src_3d
                    while w > 128:
                        dst = scr.tile([P, group, w], F16, tag=f"fold{w}")
                        nc.vector.tensor_tensor(
                            dst[:, 0:g, :], src[:, :, 0:w], src[:, :, w : 2 * w],
                            op=MAX,
                        )
                        src = dst[:, 0:g, :]
                        w //= 2
                    nc.vector.tensor_tensor(
                        rowacc[:, out_slice].rearrange("p (s w) -> p s w", s=g),
                        src[:, :, 0:128],
                        src[:, :, 128:256],
                        op=MAX,
                    )

                G = group
                for ip in range(NSTRIP // G):
                    dstrip = strips.tile([P, G, N], F16, tag="strip")
                    for s in range(G):
                        i = G * ip + s
                        first = i == 0
                        for h in range(2):
                            ph = psum.tile([P, MHALF], F32, tag="ph")
                            emit_mms(i, h, ph)
                            # cast f32 PSUM -> f16 SBUF (ScalarE/ACT);
                            # strip 0 initializes colacc directly
                            if first:
                                nc.scalar.copy(
                                    colacc[:, h * MHALF : (h + 1) * MHALF], ph[:]
                                )
                            else:
                                nc.scalar.copy(
                                    dstrip[:, s, h * MHALF : (h + 1) * MHALF], ph[:]
                                )
                        if not first:
                            nc.vector.tensor_tensor(
                                colacc[:], colacc[:], dstrip[:, s, :], op=MAX
                            )
                    if ip == 0:
                        # strip 0 rowfold reads colacc (its cast target);
                        # runs before strips 1..3 overwrite colacc (WAR dep)
                        fold_tree(
                            colacc[:].rearrange("p (s w) -> p s w", s=1),
                            1, slice(0, 128),
                        )
                        fold_tree(dstrip[:, 1:G, :], G - 1, slice(128, G * 128))
                    else:
                        fold_tree(
                            dstrip[:], G,
                            slice(G * ip * 128, (G * ip + G) * 128),
                        )

                # ---- tail ----
                # summ[:, 0:32]  = per-(partition, strip) rowmax
                # summ[:, 32:64] = per-(partition, block) colmax via PE transpose
                rw = 64
                v = rowacc[:].rearrange("p (i w) -> p i w", w=128)
                while rw >= 2:
                    rdst = scr.tile([P, NSTRIP, rw], F16, tag=f"rfold{rw}")
                    nc.vector.tensor_tensor(
                        rdst[:], v[:, :, 0:rw], v[:, :, rw : 2 * rw], op=MAX
                    )
                    v = rdst[:]
                    rw //= 2
                nc.vector.tensor_reduce(
                    out=summ[:, 0:NSTRIP],
                    in_=v,
                    axis=mybir.AxisListType.X,
                    op=MAX,
                )
                # colacc partition-max: 8 transposes per 1-bank PSUM tile,
                # one grouped 3D reduce per tile (4 DVE ops total)
                TG = 8
                for q in range(NSTRIP // TG):
                    tp = psum.tile([P, TG, P], F16, tag="ph")
                    for j in range(TG):
                        k = TG * q + j
                        nc.tensor.transpose(
                            tp[:, j, :], colacc[:, k * P : (k + 1) * P], idt[:]
                        )
                    nc.vector.tensor_reduce(
                        out=summ[:, NSTRIP + TG * q : NSTRIP + TG * q + TG],
                        in_=tp[:],
                        axis=mybir.AxisListType.X,
                        op=MAX,
                    )
                tot = accs.tile([P, 1], F32)
                nc.vector.tensor_reduce(
                    out=tot[:], in_=summ[:], axis=mybir.AxisListType.X, op=ADD
                )
                # partition-sum on PE: ones[128,1]^T @ tot[128,1] -> [1,1]
                ptot = psum.tile([1, 1], F32, tag="ph")
                nc.tensor.matmul(
                    ptot[:], lhsT=ones[:], rhs=tot[:], start=True, stop=True
                )
                res = accs.tile([1, 1], F32)
                nc.scalar.copy(res[:], ptot[:])
                nc.sync.dma_start(out=out[:], in_=res[:])

    nc.compile()
    return nc


def get_nc(repeat=1, packed=True, gsplit=0, group=4, maskred=False):
    key = (repeat, packed, gsplit, group, maskred)
    if key not in _NC_CACHE:
        _NC_CACHE[key] = _build_nc(
            repeat=repeat, packed=packed, gsplit=gsplit, group=group,
            maskred=maskred,
        )
    return _NC_CACHE[key]


def _lift(points1, points2):
    """Host-side O(N) prep: lifted vectors so -dist = la^T @ lb."""
    p1 = np.asarray(points1, dtype=np.float32)
    p2 = np.asarray(points2, dtype=np.float32)
    sq1 = np.sum(p1 * p1, axis=-1)  # [B, N]
    sq2 = np.sum(p2 * p2, axis=-1)  # [B, N]
    la = np.zeros((B, K, N), dtype=np.float32)
    lb = np.zeros((B, K, N), dtype=np.float32)
    la[:, 0, :] = sq1
    la[:, 1, :] = 1.0
    la[:, 2:5, :] = np.transpose(p1, (0, 2, 1))
    lb[:, 0, :] = -1.0
    lb[:, 1, :] = -sq2
    lb[:, 2:5, :] = 2.0 * np.transpose(p2, (0, 2, 1))
    return la, lb


def _in_maps(points1, points2):
    la, lb = _lift(points1, points2)
    ident = np.eye(P, dtype=np.float16)
    return [
        {
            "la": np.ascontiguousarray(la[b]),
            "lb": np.ascontiguousarray(lb[b]),
            "ident": ident,
        }
        for b in range(B)
    ]


def kernel(points1, points2):
    from concourse.bass_utils import run_bass_kernel_spmd

    in_maps = _in_maps(points1, points2)
    nc = get_nc()
    res = run_bass_kernel_spmd(nc, in_maps, list(range(B))).results
    tot = -sum(float(res[b]["partial"][0, 0]) for b in range(B))
    loss = tot / (B * B * N)
    return np.float32(loss)
